# revision 13
# baseline (speedup 1.0000x reference)
"""Trainium2 Bass kernel: multi-head attention (b=4, s=2048, d_model=1024, h=16).

Sharding over 8 NeuronCores: 2-D (batch x head-half).
  core c -> batch c//2, head group c%2 (8 of 16 heads, qkv dims 512*g..512*g+512).
Per core: QKV column-parallel, per-head attention (scores computed transposed,
softmax sums via a ones-column appended to V in the PV matmul, max-subtraction
skipped -- scores are O(5) so exp is safe), a per-(head-pair, seq-quarter)
AllGather of the normalized per-head outputs, and a column-parallel output
projection computed as three passes of partial sums.

The scalar-engine exp stream (~285us) is the critical path; everything else is
arranged around it:
  - Batched startup DMAs (few triggers) + minimal upfront projection groups
    -> the first exp fires ~12us in.
  - All QKV projection groups are dripped as whole psum-groups at paced drop
    points inside the hp0/hp1 attention units; x/weights are freed at hp2.
  - The output projection is split by y row-blocks so each pass only depends
    on AllGathers that are a full head-pair old (the Tile scheduler models
    collectives pessimistically and defers anything depending on a recent
    one): pass A (hp0+hp1 blocks) drips during hp2's units into a bf16
    partial (reusing the retired qT0/kT0/qT1/kT1 SBUF, bias folded in),
    pass A2 (hp2 blocks) accumulates during hp3's units 0-2, and only
    pass B (hp3 blocks + final add) remains after the last exp.

All matmul operands are bf16 (fp32 PSUM accumulation). The host pre-transposes
x to x^T [D, S] and casts to bf16. The two score matmuls per (head-pair, k)
are row-tiled (tile_position auto-derived from base partitions 0/64) and run
concurrently on the PE.

Host assembly: out[b] = concat(core 2b cols 0:512, core 2b+1 cols 512:1024).

Self-contained: hardcodes all shapes; builds/compiles once per process.
"""

from contextlib import ExitStack

import ml_dtypes
import numpy as np

import concourse.bass as bass
import concourse.mybir as mybir
import concourse.tile as tile
from concourse import bacc
from concourse.bass_utils import run_bass_kernel_spmd

FP = mybir.dt.float32
BF = mybir.dt.bfloat16
AFT = mybir.ActivationFunctionType
ts = bass.ts

NCORES = 8
D = 1024           # d_model
HD = 64            # head dim
HPC = 8            # heads per core
DQ = HPC * HD      # per-core qkv width = 512
SCALE = 1.0 / np.sqrt(HD)


def emit_mha(nc, tc, io, S):
    """Emit the per-core MHA program. io: dict of DRAM APs."""
    NHP = HPC // 2       # head pairs = 4
    KT = S // 128        # sk tiles = 16
    SQB = S // 512       # sq blocks of 512 = 4
    DKT = D // 128       # d_in tiles = 8
    MQ = DQ // 128       # qkv dout tiles = 4
    TT = S // 128        # token tiles = 16

    xt_in, wq_in, bqk_in, wk_in, wv_in, bv_in, wo_in, bo_in, out_ext = (
        io["xt"], io["wq"], io["bqk"], io["wk"], io["wv"], io["bv"],
        io["wo"], io["bo"], io["out"])

    # DRAM sources viewed [part, k-tile, cols]
    xt_r = xt_in.rearrange("(k p) s -> p k s", p=128)
    wq_r = wq_in.rearrange("(k p) c -> p k c", p=128)
    wk_r = wk_in.rearrange("(k p) c -> p k c", p=128)
    wv_r = wv_in.rearrange("(k p) c -> p k c", p=128)
    wo_r = wo_in.rearrange("(k p) c -> p k c", p=128)

    with ExitStack() as ctx:
        const_pool = ctx.enter_context(tc.tile_pool(name="const", bufs=1))
        dram_pool = ctx.enter_context(tc.tile_pool(name="dram", bufs=1, space="DRAM"))
        # PSUM budget: mm 2 + scores 2x2 + accA 1 + accB 1 = 8 banks
        mm_psum = ctx.enter_context(
            tc.tile_pool(name="mmps", bufs=2, space="PSUM"))
        sc_psum = ctx.enter_context(
            tc.tile_pool(name="scps", bufs=2, space="PSUM"))
        ac_psum = ctx.enter_context(
            tc.tile_pool(name="acps", bufs=1, space="PSUM"))

        # biases for q/k, host-packed [128, 2*MQ]: col m = bq tile m, MQ+m = bk
        bias_qk = const_pool.tile([128, 2 * MQ], FP, tag="bqk", name="bqk")
        nc.sync.dma_start(bias_qk[:], bqk_in[:, :])

        # bv / bo broadcast tiles [128, DQ]
        bv_bc = const_pool.tile([128, DQ], FP, tag="bvbc", name="bvbc")
        bo_bc = const_pool.tile([128, DQ], FP, tag="bobc", name="bobc")
        with tc.tile_pool(name="btmpp", bufs=2) as btmp_pool:
            btmp = btmp_pool.tile([128, DQ], FP, tag="btmp", name="btmp")
            nc.sync.dma_start(
                btmp[0:1, :], bv_in[:].rearrange("(one f) -> one f", one=1))
            nc.gpsimd.partition_broadcast(bv_bc[:], btmp[0:1, :])
            btmp2 = btmp_pool.tile([128, DQ], FP, tag="btmp", name="btmp2")
            nc.sync.dma_start(
                btmp2[0:1, :], bo_in[:].rearrange("(one f) -> one f", one=1))
            nc.gpsimd.partition_broadcast(bo_bc[:], btmp2[0:1, :])

        def sum_slot(h, sqb):
            # unit (h, sqb) -> partition 32*(h%4), cols [(h//4)*SQB + sqb]*512
            return 32 * (h % 4), ts((h // 4) * SQB + sqb, 512)

        # DRAM bounce + AllGather in/out (bf16): per seq-quarter for head
        # pairs 0-2; hp3 ships per seq-HALF (its consumers all run in the
        # tail, and fewer final collectives avoids CC-core serialization)
        y_bnc = [[dram_pool.tile([128, 512], BF, tag=f"ybounce{hp}_{q}",
                                 name=f"ybounce{hp}_{q}")
                  for q in range(SQB)]
                 for hp in range(NHP - 1)]
        y_gath = [[dram_pool.tile([256, 512], BF, tag=f"ygather{hp}_{q}",
                                  name=f"ygather{hp}_{q}")
                   for q in range(SQB)]
                  for hp in range(NHP - 1)]
        y_bnc3 = [dram_pool.tile([128, 1024], BF, tag=f"ybounce3_{h}",
                                 name=f"ybounce3_{h}") for h in range(2)]
        y_gath3 = [dram_pool.tile([256, 1024], BF, tag=f"ygather3_{h}",
                                  name=f"ygather3_{h}") for h in range(2)]

        with ExitStack() as phase12:
            qkv_pool = phase12.enter_context(tc.tile_pool(name="qkv", bufs=1))
            yt_pool = phase12.enter_context(tc.tile_pool(name="yt", bufs=1))
            exp_pool = phase12.enter_context(tc.tile_pool(name="exp", bufs=6))
            stage_pool = phase12.enter_context(tc.tile_pool(name="stage", bufs=3))

            # q^T / k^T, d-major: tile hp holds heads 2hp (parts 0-63), 2hp+1
            qT = [qkv_pool.tile([128, S], BF, tag=f"qT{m}", name=f"qT{m}")
                  for m in range(MQ)]
            kT = [qkv_pool.tile([128, S], BF, tag=f"kT{m}", name=f"kT{m}")
                  for m in range(MQ)]
            # v natural [tok, dout] with a ones column per head
            v_ones = [qkv_pool.tile([128, HPC * (HD + 1)], BF, tag=f"v{t}",
                                    name=f"v{t}")
                      for t in range(TT)]
            # y^T (attention out, d-major, normalized in place per unit)
            yT = [yt_pool.tile([128, S], BF, tag=f"yT{m}", name=f"yT{m}")
                  for m in range(MQ)]
            # softmax sums / reciprocals, packed 32-partition-aligned
            sums_t = yt_pool.tile([128, 2 * SQB * 512], FP, tag="sums",
                                  name="sums")
            recip_t = yt_pool.tile([128, 2 * SQB * 512], FP, tag="recip",
                                   name="recip")
            nc.gpsimd.memset(sums_t[:], 1.0)

            phase01 = ExitStack()
            xtw_pool = phase01.enter_context(tc.tile_pool(name="xtw", bufs=1))
            xTall = xtw_pool.tile([128, DKT * S], BF, tag="xTall",
                                  name="xTall")
            xT3 = xTall[:].rearrange("p (d s) -> p d s", s=S)
            # wq/wk as per-head-pair [128, DKT*128] k-major strips
            wqm = [xtw_pool.tile([128, DKT * 128], BF, tag=f"wqm{m}",
                                 name=f"wqm{m}") for m in range(MQ)]
            wkm = [xtw_pool.tile([128, DKT * 128], BF, tag=f"wkm{m}",
                                 name=f"wkm{m}") for m in range(MQ)]
            wq3 = [t[:].rearrange("p (k c) -> p k c", c=128) for t in wqm]
            wk3 = [t[:].rearrange("p (k c) -> p k c", c=128) for t in wkm]
            wvt = xtw_pool.tile([128, DKT * DQ], BF, tag="wvt", name="wvt")
            wv3 = wvt[:].rearrange("p (k c) -> p k c", c=DQ)

            # ---- DMA order = first-consumption order, batched triggers ----
            for kp in range(2):
                nc.sync.dma_start(wk3[0][:, ts(kp, 4), :],
                                  wk_r[:, ts(kp, 4), ts(0, 128)])
            for kp in range(4):
                nc.sync.dma_start(xT3[:, ts(kp, 2), ts(0, 512)],
                                  xt_r[:, ts(kp, 2), ts(0, 512)])
            for kp in range(2):
                nc.sync.dma_start(wq3[0][:, ts(kp, 4), :],
                                  wq_r[:, ts(kp, 4), ts(0, 128)])
            for kp in range(2):
                nc.sync.dma_start(wv3[:, ts(kp, 4), :],
                                  wv_r[:, ts(kp, 4), :])
            for nb in range(1, SQB):
                for kp in range(4):
                    nc.sync.dma_start(xT3[:, ts(kp, 2), ts(nb, 512)],
                                      xt_r[:, ts(kp, 2), ts(nb, 512)])
            for m in range(1, MQ):
                nc.sync.dma_start(wk3[m][:, :, :], wk_r[:, :, ts(m, 128)])
                nc.sync.dma_start(wq3[m][:, :, :], wq_r[:, :, ts(m, 128)])

            def emit_qk_group(hp, which, nb):
                # one q/k projection psum group for head-pair hp; which: 0=q,
                # 1=k; nb = token block. Evac on DVE (keeps the scalar engine
                # free for the exp stream).
                w3, dstT = ((wq3, qT), (wk3, kT))[which]
                ps = mm_psum.tile([128, 512], FP, tag="mm", name="mm")
                for k in range(DKT):
                    nc.tensor.matmul(
                        ps[:], lhsT=w3[hp][:, k, :], rhs=xT3[:, k, ts(nb, 512)],
                        start=(k == 0), stop=(k == DKT - 1))
                col = which * MQ + hp
                nc.vector.tensor_scalar_add(
                    dstT[hp][:, ts(nb, 512)], ps[:],
                    bias_qk[:, col:col + 1])

            def emit_v(ti):
                ps = mm_psum.tile([128, DQ], FP, tag="mm", name="mm")
                for k in range(DKT):
                    nc.tensor.matmul(
                        ps[:], lhsT=xT3[:, k, ts(ti, 128)], rhs=wv3[:, k, :],
                        start=(k == 0), stop=(k == DKT - 1))
                vt3 = v_ones[ti][:].rearrange("p (h u) -> p h u", u=HD + 1)
                nc.vector.tensor_add(
                    vt3[:, :, 0:HD],
                    ps[:].rearrange("p (h u) -> p h u", u=HD),
                    bv_bc[:].rearrange("p (h u) -> p h u", u=HD))
                nc.gpsimd.memset(vt3[:, :, HD:HD + 1], 1.0)

            # ---- upfront: the two groups unit (0,0) needs to start ----
            emit_qk_group(0, 1, 0)   # k proj, head-pair 0, tokens 0:512
            emit_qk_group(0, 0, 0)   # q proj, head-pair 0, tokens 0:512

            # ---- paced drip schedule: {(hp,sqb): {iter: [fn]}} ----
            drip = {}

            def add_drop(hp, sqb, it, fn):
                drip.setdefault((hp, sqb), {}).setdefault(it, []).append(fn)

            def qk(hp, w, nb):
                return lambda: emit_qk_group(hp, w, nb)

            # remaining hp0 groups in unit (0,0); hp1's during hp0's units
            for i, (w, nb) in enumerate(((1, 1), (1, 2), (1, 3), (0, 1))):
                add_drop(0, 0, 3 * i + 1, qk(0, w, nb))
            sched01 = [
                ((0, 1), ((0, 0, 2), (1, 1, 0), (1, 1, 1))),
                ((0, 2), ((0, 0, 3), (1, 1, 2), (1, 1, 3))),
                ((0, 3), ((1, 0, 0), (1, 0, 1), (1, 0, 2), (1, 0, 3))),
                ((1, 0), ((2, 1, 0), (2, 1, 1), (2, 1, 2), (2, 1, 3))),
                ((1, 1), ((2, 0, 0), (2, 0, 1), (2, 0, 2), (2, 0, 3))),
                ((1, 2), ((3, 1, 0), (3, 1, 1), (3, 1, 2), (3, 1, 3))),
                ((1, 3), ((3, 0, 0), (3, 0, 1), (3, 0, 2), (3, 0, 3))),
            ]
            for (hp, sqb), groups in sched01:
                step = 4 if len(groups) == 4 else 5
                for i, (ghp, w, nb) in enumerate(groups):
                    add_drop(hp, sqb, step * i + 1, qk(ghp, w, nb))

            # ---- output projection: three passes of y row-blocks ----
            # k2 in 0..3 -> rank0 rows of gather hp=k2 (y_full rows 128*k2);
            # k2 in 4..7 -> rank1 rows of gather hp=k2-4.
            wot3_ref = [None]
            ygq_pool_ref = [None]
            out_stage_ref = [None]
            p1t = [None] * MQ      # bf16 partials, reuse retired qT/kT space
            ygq = {}               # (k2, q) -> sbuf tile

            def p1(ti):
                return p1t[ti // 4][:, ts(ti % 4, 512)]

            def load_wo():
                p4_pool = phase12.enter_context(
                    tc.tile_pool(name="p4", bufs=1))
                ygq_pool_ref[0] = phase12.enter_context(
                    tc.tile_pool(name="ygq", bufs=4))
                out_stage_ref[0] = phase12.enter_context(
                    tc.tile_pool(name="outp", bufs=3))
                wot = p4_pool.tile([128, 2 * MQ * DQ], BF, tag="wot",
                                   name="wot")
                wot3_ref[0] = wot[:].rearrange("p (k c) -> p k c", c=DQ)
                for kp in range(2):
                    nc.sync.dma_start(wot3_ref[0][:, ts(kp, 4), :],
                                      wo_r[:, ts(kp, 4), :])
                for g in range(MQ):
                    tag = (f"qT{g // 2}", f"kT{g // 2}")[g % 2]
                    p1t[g] = qkv_pool.tile([128, S], BF, tag=tag,
                                           name=f"p1_{g}")

            def load_ygq(q, k2s, eng=None):
                for k2 in k2s:
                    t = ygq_pool_ref[0].tile([128, 512], BF, tag=f"yg{k2}",
                                             name=f"yg{k2}_{q}")
                    half = slice(0, 128) if k2 < MQ else slice(128, 256)
                    if k2 % MQ == 3:
                        (eng or nc.sync).dma_start(
                            t[:], y_gath3[q // 2][half, ts(q % 2, 512)])
                    else:
                        (eng or nc.sync).dma_start(
                            t[:], y_gath[k2 % MQ][q][half, :])
                    ygq[(k2, q)] = t

            def outproj_pass(ti, k2s, first):
                po = mm_psum.tile([128, DQ], FP, tag="mm", name="mm")
                for j, k2 in enumerate(k2s):
                    nc.tensor.matmul(
                        po[:], lhsT=ygq[(k2, ti // 4)][:, ts(ti % 4, 128)],
                        rhs=wot3_ref[0][:, k2, :],
                        start=(j == 0), stop=(j == len(k2s) - 1))
                if first:
                    nc.vector.tensor_add(p1(ti), po[:], bo_bc[:])
                else:
                    nc.vector.tensor_add(p1(ti), p1(ti), po[:])

            PA_K2, PB_K2 = (0, 1, 4, 5), (2, 3, 6, 7)
            for q in range(SQB):
                add_drop(2, q, 0, lambda q=q: load_ygq(q, PA_K2))
                for tl in range(4):
                    add_drop(2, q, 4 * tl + 1,
                             lambda ti=4 * q + tl:
                                 outproj_pass(ti, PA_K2, True))
            # prefetch hp2's gathered-y quarters during hp3 (DMA only; the
            # matmuls run in the tail pass to keep hp3's DVE queue clean)
            for q in range(SQB):
                add_drop(3, min(q, 2), 4 * q + 1,
                         lambda q=q: load_ygq(q, (2, 6)))

            # ---- attention units ----
            for hp in range(NHP):
                if hp == 2:
                    # all projection drips done -> free x/weights, bring in Wo
                    phase01.close()
                    load_wo()

                hA, hB = 2 * hp, 2 * hp + 1
                for sqb in range(SQB):
                    sq = ts(sqb, 512)
                    drops = drip.get((hp, sqb), {})
                    for fn in drops.get(-1, ()):
                        fn()
                    accA = ac_psum.tile([HD + 1, 512], FP, tag="accA",
                                        name="accA")
                    accB = ac_psum.tile([HD + 1, 512], FP, tag="accB",
                                        name="accB")

                    def emit_scores(k):
                        sk = ts(k, 128)
                        ps = sc_psum.tile([128, 1024], FP, tag="sc",
                                          name="sc")
                        # scores^T [sk, sq] for both heads; base partitions
                        # 0/64 -> row-tiled, the matmuls run concurrently
                        nc.tensor.matmul(
                            ps[:, 0:512], lhsT=kT[hp][0:64, sk],
                            rhs=qT[hp][0:64, sq], start=True, stop=True)
                        nc.tensor.matmul(
                            ps[:, 512:1024], lhsT=kT[hp][64:128, sk],
                            rhs=qT[hp][64:128, sq], start=True, stop=True)
                        et = exp_pool.tile([128, 1024], BF, tag="exp",
                                           name="exp")
                        nc.scalar.activation(et[:], ps[:], AFT.Exp,
                                             scale=SCALE)
                        if hp == 0 and sqb == 0:
                            # produce v[k] just in time for its attnv
                            emit_v(k)
                        return et

                    def emit_av(k, et):
                        # y^T accumulation: lhsT = [v_h | 1]
                        nc.tensor.matmul(
                            accA[:], lhsT=v_ones[k][:, hA * 65:hA * 65 + 65],
                            rhs=et[:, 0:512],
                            start=(k == 0), stop=(k == KT - 1),
                            skip_group_check=True)
                        nc.tensor.matmul(
                            accB[:], lhsT=v_ones[k][:, hB * 65:hB * 65 + 65],
                            rhs=et[:, 512:1024],
                            start=(k == 0), stop=(k == KT - 1),
                            skip_group_check=True)
                        for fn in drops.get(k, ()):
                            fn()

                    # software-pipeline: scores run one iteration ahead of
                    # the AV accumulation, so an AV stall (e.g. the previous
                    # unit's accumulator extraction) never blocks the next
                    # score pair in the in-order PE queue -> the exp stream
                    # keeps flowing across unit boundaries.
                    prev_et = None
                    for k in range(KT):
                        et = emit_scores(k)
                        if prev_et is not None:
                            emit_av(k - 1, prev_et)
                        prev_et = et
                    emit_av(KT - 1, prev_et)
                    # extract y (rows 0-63) and sums (row 64)
                    nc.vector.tensor_copy(yT[hp][0:64, sq], accA[0:64, :])
                    st = stage_pool.tile([128, 512], BF, tag="bst", name="bst")
                    nc.vector.tensor_copy(st[0:64, :], accB[0:64, :])
                    nc.sync.dma_start(yT[hp][64:128, sq], st[0:64, :])
                    for acc, h in ((accA, hA), (accB, hB)):
                        sp, sc = sum_slot(h, sqb)
                        sA = stage_pool.tile([128, 512], FP, tag="sst",
                                             name="sst")
                        nc.vector.tensor_copy(sA[64:65, :], acc[64:65, :])
                        nc.sync.dma_start(sums_t[sp:sp + 1, sc],
                                          sA[64:65, :])
                    # reciprocal for this unit (both heads share a
                    # 64-partition band and column slot)
                    band = 32 * (hA % 4)
                    _, sc = sum_slot(hA, sqb)
                    nc.vector.reciprocal(
                        recip_t[band:band + 64, sc],
                        sums_t[band:band + 64, sc])
                    # normalize y^T for this unit in place
                    for h2, h in ((0, hA), (1, hB)):
                        rows = slice(64 * h2, 64 * h2 + 64)
                        sp, _ = sum_slot(h, sqb)
                        # HW partition_broadcast reads partition 0 of the
                        # tensor regardless of the AP base -> stage the
                        # recip row to partition 0 (cast to bf16) first.
                        rtmp = stage_pool.tile([128, 512], FP, tag="rtmp",
                                               name="rtmp")
                        nc.sync.dma_start(rtmp[0:1, :],
                                          recip_t[sp:sp + 1, sc])
                        rtb = stage_pool.tile([128, 512], BF, tag="rtb",
                                              name="rtb")
                        nc.vector.tensor_copy(rtb[0:1, :], rtmp[0:1, :])
                        rb = stage_pool.tile([128, 512], BF, tag="rb",
                                             name="rb")
                        nc.gpsimd.partition_broadcast(rb[:], rtb[0:1, :])
                        nc.vector.tensor_mul(
                            yT[hp][rows, sq], yT[hp][rows, sq],
                            rb[rows, :])
                    # ship + AllGather: per quarter for hp 0-2, per half
                    # for hp3 (after sqb 1 and 3)
                    if hp < NHP - 1:
                        nc.sync.dma_start(y_bnc[hp][sqb][:, :], yT[hp][:, sq])
                        nc.gpsimd.collective_compute(
                            "AllGather", mybir.AluOpType.bypass,
                            replica_groups=[[0, 1], [2, 3], [4, 5], [6, 7]],
                            ins=[y_bnc[hp][sqb][:, :]],
                            outs=[y_gath[hp][sqb][:, :]])
                    elif sqb % 2 == 1:
                        h = sqb // 2
                        nc.sync.dma_start(y_bnc3[h][:, :],
                                          yT[hp][:, ts(h, 1024)])
                        nc.gpsimd.collective_compute(
                            "AllGather", mybir.AluOpType.bypass,
                            replica_groups=[[0, 1], [2, 3], [4, 5], [6, 7]],
                            ins=[y_bnc3[h][:, :]],
                            outs=[y_gath3[h][:, :]])

            # ---- tail: hp3 row-blocks of the output projection ----
            # tail DMAs trigger from the Scalar queue (idle after the last
            # exp) so the Sync queue stays clear for the final units'
            # normalize -> ship -> AllGather chains
            for q in range(SQB):
                load_ygq(q, (3, 7), eng=nc.scalar)
                for tl in range(4):
                    ti = 4 * q + tl
                    po = mm_psum.tile([128, DQ], FP, tag="mm", name="mm")
                    for j, k2 in enumerate(PB_K2):
                        nc.tensor.matmul(
                            po[:], lhsT=ygq[(k2, q)][:, ts(tl, 128)],
                            rhs=wot3_ref[0][:, k2, :],
                            start=(j == 0), stop=(j == len(PB_K2) - 1))
                    ot = out_stage_ref[0].tile([128, DQ], FP, tag="ot",
                                               name="ot")
                    nc.vector.tensor_add(ot[:], po[:], p1(ti))
                    nc.scalar.dma_start(out_ext[ts(ti, 128), :], ot[:])


def build_program(S=2048):
    nc = bacc.Bacc(
        "TRN2",
        target_bir_lowering=False,
        debug=False,
        enable_asserts=True,
        num_devices=NCORES,
    )
    io = {
        "xt": nc.declare_dram_parameter("xt", [D, S], BF, isOutput=False),
        "wq": nc.declare_dram_parameter("wq", [D, DQ], BF, isOutput=False),
        "bqk": nc.declare_dram_parameter("bqk", [128, 8], FP, isOutput=False),
        "wk": nc.declare_dram_parameter("wk", [D, DQ], BF, isOutput=False),
        "wv": nc.declare_dram_parameter("wv", [D, DQ], BF, isOutput=False),
        "bv": nc.declare_dram_parameter("bv", [DQ], FP, isOutput=False),
        "wo": nc.declare_dram_parameter("wo", [D, DQ], BF, isOutput=False),
        "bo": nc.declare_dram_parameter("bo", [DQ], FP, isOutput=False),
        "out": nc.declare_dram_parameter("out", [S, DQ], FP, isOutput=True),
    }
    io = {k: (v[:] if not isinstance(v, bass.AP) else v) for k, v in io.items()}
    with tile.TileContext(nc) as tc:
        emit_mha(nc, tc, io, S)
    nc.finalize()
    return nc


def shard_inputs(x, Wq, bq, Wk, bk, Wv, bv, Wo, bo):
    """Full inputs -> per-core in_maps. Matmul operands cast to bf16; x is
    transposed on the host (input prep for the d-major device layout)."""
    BFNP = ml_dtypes.bfloat16
    f32 = lambda a: np.ascontiguousarray(np.asarray(a), dtype=np.float32)
    bf = lambda a: np.ascontiguousarray(np.asarray(a, dtype=np.float32)
                                        .astype(BFNP))
    x = np.asarray(x, dtype=np.float32).astype(BFNP)
    xts = [np.ascontiguousarray(x[b].T) for b in range(4)]
    Wq, Wk, Wv, Wo = bf(Wq), bf(Wk), bf(Wv), bf(Wo)
    bq, bk, bv, bo = f32(bq), f32(bk), f32(bv), f32(bo)
    in_maps = []
    for c in range(NCORES):
        b, g = divmod(c, 2)
        sl = slice(g * DQ, (g + 1) * DQ)
        bqk = np.empty((128, 8), np.float32)
        for m in range(4):
            bqk[:, m] = bq[sl][m * 128:(m + 1) * 128]
            bqk[:, 4 + m] = bk[sl][m * 128:(m + 1) * 128]
        in_maps.append({
            "xt": xts[b],
            "wq": np.ascontiguousarray(Wq[:, sl]), "bqk": bqk,
            "wk": np.ascontiguousarray(Wk[:, sl]),
            "wv": np.ascontiguousarray(Wv[:, sl]), "bv": bv[sl].copy(),
            "wo": np.ascontiguousarray(Wo[:, sl]), "bo": bo[sl].copy(),
        })
    return in_maps


_CACHE = {}


def _get_program(S=2048):
    if S not in _CACHE:
        _CACHE[S] = build_program(S)
    return _CACHE[S]


def kernel(x, Wq, bq, Wk, bk, Wv, bv, Wo, bo):
    nc = _get_program(2048)
    in_maps = shard_inputs(x, Wq, bq, Wk, bk, Wv, bv, Wo, bo)
    res = run_bass_kernel_spmd(nc, in_maps, list(range(NCORES))).results
    S = 2048
    out = np.empty((4, S, D), dtype=np.float32)
    for c in range(NCORES):
        b, g = divmod(c, 2)
        out[b, :, g * DQ:(g + 1) * DQ] = res[c]["out"]
    return out


# revision 14
# speedup vs baseline: 1.0133x; 1.0133x over previous
"""Trainium2 Bass kernel: multi-head attention (b=4, s=2048, d_model=1024, h=16).

Sharding over 8 NeuronCores: 2-D (batch x head-half).
  core c -> batch c//2, head group c%2 (8 of 16 heads, qkv dims 512*g..512*g+512).
Per core: QKV column-parallel, per-head attention (scores computed transposed,
softmax sums via a ones-column appended to V in the PV matmul, max-subtraction
skipped -- scores are O(5) so exp is safe), a per-(head-pair, seq-quarter)
AllGather of the normalized per-head outputs, and a column-parallel output
projection computed as three passes of partial sums.

The scalar-engine exp stream (~285us) is the critical path; everything else is
arranged around it:
  - Batched startup DMAs (few triggers) + minimal upfront projection groups
    -> the first exp fires ~12us in.
  - All QKV projection groups are dripped as whole psum-groups at paced drop
    points inside the hp0/hp1 attention units; x/weights are freed at hp2.
  - The output projection is split by y row-blocks so each pass only depends
    on AllGathers that are a full head-pair old (the Tile scheduler models
    collectives pessimistically and defers anything depending on a recent
    one): pass A (hp0+hp1 blocks) drips during hp2's units into a bf16
    partial (reusing the retired qT0/kT0/qT1/kT1 SBUF, bias folded in),
    pass A2 (hp2 blocks) accumulates during hp3's units 0-2, and only
    pass B (hp3 blocks + final add) remains after the last exp.

All matmul operands are bf16 (fp32 PSUM accumulation). The host pre-transposes
x to x^T [D, S] and casts to bf16. The two score matmuls per (head-pair, k)
are row-tiled (tile_position auto-derived from base partitions 0/64) and run
concurrently on the PE.

Host assembly: out[b] = concat(core 2b cols 0:512, core 2b+1 cols 512:1024).

Self-contained: hardcodes all shapes; builds/compiles once per process.
"""

from contextlib import ExitStack

import ml_dtypes
import numpy as np

import concourse.bass as bass
import concourse.mybir as mybir
import concourse.tile as tile
from concourse import bacc
from concourse.bass_utils import run_bass_kernel_spmd

FP = mybir.dt.float32
BF = mybir.dt.bfloat16
AFT = mybir.ActivationFunctionType
ts = bass.ts

NCORES = 8
D = 1024           # d_model
HD = 64            # head dim
HPC = 8            # heads per core
DQ = HPC * HD      # per-core qkv width = 512
SCALE = 1.0 / np.sqrt(HD)


def emit_mha(nc, tc, io, S):
    """Emit the per-core MHA program. io: dict of DRAM APs."""
    NHP = HPC // 2       # head pairs = 4
    KT = S // 128        # sk tiles = 16
    SQB = S // 512       # sq blocks of 512 = 4
    DKT = D // 128       # d_in tiles = 8
    MQ = DQ // 128       # qkv dout tiles = 4
    TT = S // 128        # token tiles = 16

    xt_in, wq_in, bqk_in, wk_in, wv_in, bv_in, wo_in, bo_in, out_ext = (
        io["xt"], io["wq"], io["bqk"], io["wk"], io["wv"], io["bv"],
        io["wo"], io["bo"], io["out"])

    # DRAM sources viewed [part, k-tile, cols]
    xt_r = xt_in.rearrange("(k p) s -> p k s", p=128)
    wq_r = wq_in.rearrange("(k p) c -> p k c", p=128)
    wk_r = wk_in.rearrange("(k p) c -> p k c", p=128)
    wv_r = wv_in.rearrange("(k p) c -> p k c", p=128)
    wo_r = wo_in.rearrange("(k p) c -> p k c", p=128)

    with ExitStack() as ctx:
        const_pool = ctx.enter_context(tc.tile_pool(name="const", bufs=1))
        dram_pool = ctx.enter_context(tc.tile_pool(name="dram", bufs=1, space="DRAM"))
        # PSUM budget: mm 2 + scores 2x2 + accA 1 + accB 1 = 8 banks
        mm_psum = ctx.enter_context(
            tc.tile_pool(name="mmps", bufs=2, space="PSUM"))
        sc_psum = ctx.enter_context(
            tc.tile_pool(name="scps", bufs=2, space="PSUM"))
        ac_psum = ctx.enter_context(
            tc.tile_pool(name="acps", bufs=1, space="PSUM"))

        # biases for q/k, host-packed [128, 2*MQ]: col m = bq tile m, MQ+m = bk
        bias_qk = const_pool.tile([128, 2 * MQ], FP, tag="bqk", name="bqk")
        nc.sync.dma_start(bias_qk[:], bqk_in[:, :])

        # bv / bo broadcast tiles [128, DQ]
        bv_bc = const_pool.tile([128, DQ], FP, tag="bvbc", name="bvbc")
        bo_bc = const_pool.tile([128, DQ], FP, tag="bobc", name="bobc")
        with tc.tile_pool(name="btmpp", bufs=2) as btmp_pool:
            btmp = btmp_pool.tile([128, DQ], FP, tag="btmp", name="btmp")
            nc.sync.dma_start(
                btmp[0:1, :], bv_in[:].rearrange("(one f) -> one f", one=1))
            nc.gpsimd.partition_broadcast(bv_bc[:], btmp[0:1, :])
            btmp2 = btmp_pool.tile([128, DQ], FP, tag="btmp", name="btmp2")
            nc.sync.dma_start(
                btmp2[0:1, :], bo_in[:].rearrange("(one f) -> one f", one=1))
            nc.gpsimd.partition_broadcast(bo_bc[:], btmp2[0:1, :])

        def sum_slot(h, sqb):
            # unit (h, sqb) -> partition 32*(h%4), cols [(h//4)*SQB + sqb]*512
            return 32 * (h % 4), ts((h // 4) * SQB + sqb, 512)

        # DRAM bounce + per-(head-pair, seq-quarter) AllGather in/out (bf16)
        y_bnc = [[dram_pool.tile([128, 512], BF, tag=f"ybounce{hp}_{q}",
                                 name=f"ybounce{hp}_{q}")
                  for q in range(SQB)]
                 for hp in range(NHP)]
        y_gath = [[dram_pool.tile([256, 512], BF, tag=f"ygather{hp}_{q}",
                                  name=f"ygather{hp}_{q}")
                   for q in range(SQB)]
                  for hp in range(NHP)]

        with ExitStack() as phase12:
            qkv_pool = phase12.enter_context(tc.tile_pool(name="qkv", bufs=1))
            yt_pool = phase12.enter_context(tc.tile_pool(name="yt", bufs=1))
            exp_pool = phase12.enter_context(tc.tile_pool(name="exp", bufs=6))
            stage_pool = phase12.enter_context(tc.tile_pool(name="stage", bufs=3))

            # q^T / k^T, d-major: tile hp holds heads 2hp (parts 0-63), 2hp+1
            qT = [qkv_pool.tile([128, S], BF, tag=f"qT{m}", name=f"qT{m}")
                  for m in range(MQ)]
            kT = [qkv_pool.tile([128, S], BF, tag=f"kT{m}", name=f"kT{m}")
                  for m in range(MQ)]
            # v natural [tok, dout] with a ones column per head
            v_ones = [qkv_pool.tile([128, HPC * (HD + 1)], BF, tag=f"v{t}",
                                    name=f"v{t}")
                      for t in range(TT)]
            # y^T (attention out, d-major, normalized in place per unit)
            yT = [yt_pool.tile([128, S], BF, tag=f"yT{m}", name=f"yT{m}")
                  for m in range(MQ)]
            # softmax sums / reciprocals, packed 32-partition-aligned
            sums_t = yt_pool.tile([128, 2 * SQB * 512], FP, tag="sums",
                                  name="sums")
            recip_t = yt_pool.tile([128, 2 * SQB * 512], FP, tag="recip",
                                   name="recip")
            nc.gpsimd.memset(sums_t[:], 1.0)

            phase01 = ExitStack()
            xtw_pool = phase01.enter_context(tc.tile_pool(name="xtw", bufs=1))
            xTall = xtw_pool.tile([128, DKT * S], BF, tag="xTall",
                                  name="xTall")
            xT3 = xTall[:].rearrange("p (d s) -> p d s", s=S)
            # wq/wk as per-head-pair [128, DKT*128] k-major strips
            wqm = [xtw_pool.tile([128, DKT * 128], BF, tag=f"wqm{m}",
                                 name=f"wqm{m}") for m in range(MQ)]
            wkm = [xtw_pool.tile([128, DKT * 128], BF, tag=f"wkm{m}",
                                 name=f"wkm{m}") for m in range(MQ)]
            wq3 = [t[:].rearrange("p (k c) -> p k c", c=128) for t in wqm]
            wk3 = [t[:].rearrange("p (k c) -> p k c", c=128) for t in wkm]
            wvt = xtw_pool.tile([128, DKT * DQ], BF, tag="wvt", name="wvt")
            wv3 = wvt[:].rearrange("p (k c) -> p k c", c=DQ)

            # ---- DMA order = first-consumption order, batched triggers ----
            for kp in range(2):
                nc.sync.dma_start(wk3[0][:, ts(kp, 4), :],
                                  wk_r[:, ts(kp, 4), ts(0, 128)])
            for kp in range(4):
                nc.sync.dma_start(xT3[:, ts(kp, 2), ts(0, 512)],
                                  xt_r[:, ts(kp, 2), ts(0, 512)])
            for kp in range(2):
                nc.sync.dma_start(wq3[0][:, ts(kp, 4), :],
                                  wq_r[:, ts(kp, 4), ts(0, 128)])
            for kp in range(2):
                nc.sync.dma_start(wv3[:, ts(kp, 4), :],
                                  wv_r[:, ts(kp, 4), :])
            for nb in range(1, SQB):
                for kp in range(4):
                    nc.sync.dma_start(xT3[:, ts(kp, 2), ts(nb, 512)],
                                      xt_r[:, ts(kp, 2), ts(nb, 512)])
            for m in range(1, MQ):
                nc.sync.dma_start(wk3[m][:, :, :], wk_r[:, :, ts(m, 128)])
                nc.sync.dma_start(wq3[m][:, :, :], wq_r[:, :, ts(m, 128)])

            def emit_qk_group(hp, which, nb):
                # one q/k projection psum group for head-pair hp; which: 0=q,
                # 1=k; nb = token block. Evac on DVE (keeps the scalar engine
                # free for the exp stream).
                w3, dstT = ((wq3, qT), (wk3, kT))[which]
                ps = mm_psum.tile([128, 512], FP, tag="mm", name="mm")
                for k in range(DKT):
                    nc.tensor.matmul(
                        ps[:], lhsT=w3[hp][:, k, :], rhs=xT3[:, k, ts(nb, 512)],
                        start=(k == 0), stop=(k == DKT - 1))
                col = which * MQ + hp
                nc.vector.tensor_scalar_add(
                    dstT[hp][:, ts(nb, 512)], ps[:],
                    bias_qk[:, col:col + 1])

            def emit_v(ti):
                ps = mm_psum.tile([128, DQ], FP, tag="mm", name="mm")
                for k in range(DKT):
                    nc.tensor.matmul(
                        ps[:], lhsT=xT3[:, k, ts(ti, 128)], rhs=wv3[:, k, :],
                        start=(k == 0), stop=(k == DKT - 1))
                vt3 = v_ones[ti][:].rearrange("p (h u) -> p h u", u=HD + 1)
                nc.vector.tensor_add(
                    vt3[:, :, 0:HD],
                    ps[:].rearrange("p (h u) -> p h u", u=HD),
                    bv_bc[:].rearrange("p (h u) -> p h u", u=HD))
                nc.gpsimd.memset(vt3[:, :, HD:HD + 1], 1.0)

            # ---- upfront: the two groups unit (0,0) needs to start ----
            emit_qk_group(0, 1, 0)   # k proj, head-pair 0, tokens 0:512
            emit_qk_group(0, 0, 0)   # q proj, head-pair 0, tokens 0:512

            # ---- paced drip schedule: {(hp,sqb): {iter: [fn]}} ----
            drip = {}

            def add_drop(hp, sqb, it, fn):
                drip.setdefault((hp, sqb), {}).setdefault(it, []).append(fn)

            def qk(hp, w, nb):
                return lambda: emit_qk_group(hp, w, nb)

            # remaining hp0 groups in unit (0,0); hp1's during hp0's units
            for i, (w, nb) in enumerate(((1, 1), (1, 2), (1, 3), (0, 1))):
                add_drop(0, 0, 3 * i + 1, qk(0, w, nb))
            sched01 = [
                ((0, 1), ((0, 0, 2), (1, 1, 0), (1, 1, 1))),
                ((0, 2), ((0, 0, 3), (1, 1, 2), (1, 1, 3))),
                ((0, 3), ((1, 0, 0), (1, 0, 1), (1, 0, 2), (1, 0, 3))),
                ((1, 0), ((2, 1, 0), (2, 1, 1), (2, 1, 2), (2, 1, 3))),
                ((1, 1), ((2, 0, 0), (2, 0, 1), (2, 0, 2), (2, 0, 3))),
                ((1, 2), ((3, 1, 0), (3, 1, 1), (3, 1, 2), (3, 1, 3))),
                ((1, 3), ((3, 0, 0), (3, 0, 1), (3, 0, 2), (3, 0, 3))),
            ]
            for (hp, sqb), groups in sched01:
                step = 4 if len(groups) == 4 else 5
                for i, (ghp, w, nb) in enumerate(groups):
                    add_drop(hp, sqb, step * i + 1, qk(ghp, w, nb))

            # ---- output projection: three passes of y row-blocks ----
            # k2 in 0..3 -> rank0 rows of gather hp=k2 (y_full rows 128*k2);
            # k2 in 4..7 -> rank1 rows of gather hp=k2-4.
            wot3_ref = [None]
            ygq_pool_ref = [None]
            out_stage_ref = [None]
            p1t = [None] * MQ      # bf16 partials, reuse retired qT/kT space
            ygq = {}               # (k2, q) -> sbuf tile

            def p1(ti):
                return p1t[ti // 4][:, ts(ti % 4, 512)]

            def load_wo():
                p4_pool = phase12.enter_context(
                    tc.tile_pool(name="p4", bufs=1))
                ygq_pool_ref[0] = phase12.enter_context(
                    tc.tile_pool(name="ygq", bufs=4))
                out_stage_ref[0] = phase12.enter_context(
                    tc.tile_pool(name="outp", bufs=3))
                wot = p4_pool.tile([128, 2 * MQ * DQ], BF, tag="wot",
                                   name="wot")
                wot3_ref[0] = wot[:].rearrange("p (k c) -> p k c", c=DQ)
                for kp in range(2):
                    nc.sync.dma_start(wot3_ref[0][:, ts(kp, 4), :],
                                      wo_r[:, ts(kp, 4), :])
                for g in range(MQ):
                    tag = (f"qT{g // 2}", f"kT{g // 2}")[g % 2]
                    p1t[g] = qkv_pool.tile([128, S], BF, tag=tag,
                                           name=f"p1_{g}")

            def load_ygq(q, k2s, eng=None):
                for k2 in k2s:
                    t = ygq_pool_ref[0].tile([128, 512], BF, tag=f"yg{k2}",
                                             name=f"yg{k2}_{q}")
                    src = y_gath[k2 % MQ][q]
                    half = slice(0, 128) if k2 < MQ else slice(128, 256)
                    (eng or nc.sync).dma_start(t[:], src[half, :])
                    ygq[(k2, q)] = t

            def outproj_pass(ti, k2s, first):
                po = mm_psum.tile([128, DQ], FP, tag="mm", name="mm")
                for j, k2 in enumerate(k2s):
                    nc.tensor.matmul(
                        po[:], lhsT=ygq[(k2, ti // 4)][:, ts(ti % 4, 128)],
                        rhs=wot3_ref[0][:, k2, :],
                        start=(j == 0), stop=(j == len(k2s) - 1))
                if first:
                    nc.vector.tensor_add(p1(ti), po[:], bo_bc[:])
                else:
                    nc.vector.tensor_add(p1(ti), p1(ti), po[:])

            PA_K2, PB_K2 = (0, 1, 4, 5), (2, 3, 6, 7)
            for q in range(SQB):
                add_drop(2, q, 0, lambda q=q: load_ygq(q, PA_K2))
                for tl in range(4):
                    add_drop(2, q, 4 * tl + 1,
                             lambda ti=4 * q + tl:
                                 outproj_pass(ti, PA_K2, True))
            # prefetch hp2's gathered-y quarters during hp3 (DMA only; the
            # matmuls run in the tail pass to keep hp3's DVE queue clean)
            for q in range(SQB):
                add_drop(3, min(q, 2), 4 * q + 1,
                         lambda q=q: load_ygq(q, (2, 6)))

            # ---- attention units ----
            for hp in range(NHP):
                if hp == 2:
                    # all projection drips done -> free x/weights, bring in Wo
                    phase01.close()
                    load_wo()

                hA, hB = 2 * hp, 2 * hp + 1
                for sqb in range(SQB):
                    sq = ts(sqb, 512)
                    drops = drip.get((hp, sqb), {})
                    for fn in drops.get(-1, ()):
                        fn()
                    accA = ac_psum.tile([HD + 1, 512], FP, tag="accA",
                                        name="accA")
                    accB = ac_psum.tile([HD + 1, 512], FP, tag="accB",
                                        name="accB")

                    def emit_scores(k):
                        sk = ts(k, 128)
                        ps = sc_psum.tile([128, 1024], FP, tag="sc",
                                          name="sc")
                        # scores^T [sk, sq] for both heads; base partitions
                        # 0/64 -> row-tiled, the matmuls run concurrently
                        nc.tensor.matmul(
                            ps[:, 0:512], lhsT=kT[hp][0:64, sk],
                            rhs=qT[hp][0:64, sq], start=True, stop=True)
                        nc.tensor.matmul(
                            ps[:, 512:1024], lhsT=kT[hp][64:128, sk],
                            rhs=qT[hp][64:128, sq], start=True, stop=True)
                        et = exp_pool.tile([128, 1024], BF, tag="exp",
                                           name="exp")
                        nc.scalar.activation(et[:], ps[:], AFT.Exp,
                                             scale=SCALE)
                        if hp == 0 and sqb == 0:
                            # produce v[k] just in time for its attnv
                            emit_v(k)
                        return et

                    def emit_av(k, et):
                        # y^T accumulation: lhsT = [v_h | 1]
                        nc.tensor.matmul(
                            accA[:], lhsT=v_ones[k][:, hA * 65:hA * 65 + 65],
                            rhs=et[:, 0:512],
                            start=(k == 0), stop=(k == KT - 1),
                            skip_group_check=True)
                        nc.tensor.matmul(
                            accB[:], lhsT=v_ones[k][:, hB * 65:hB * 65 + 65],
                            rhs=et[:, 512:1024],
                            start=(k == 0), stop=(k == KT - 1),
                            skip_group_check=True)
                        for fn in drops.get(k, ()):
                            fn()

                    # software-pipeline: scores run one iteration ahead of
                    # the AV accumulation, so an AV stall (e.g. the previous
                    # unit's accumulator extraction) never blocks the next
                    # score pair in the in-order PE queue -> the exp stream
                    # keeps flowing across unit boundaries.
                    prev_et = None
                    for k in range(KT):
                        et = emit_scores(k)
                        if prev_et is not None:
                            emit_av(k - 1, prev_et)
                        prev_et = et
                    emit_av(KT - 1, prev_et)
                    # extract y (rows 0-63) and sums (row 64)
                    nc.vector.tensor_copy(yT[hp][0:64, sq], accA[0:64, :])
                    st = stage_pool.tile([128, 512], BF, tag="bst", name="bst")
                    nc.vector.tensor_copy(st[0:64, :], accB[0:64, :])
                    nc.sync.dma_start(yT[hp][64:128, sq], st[0:64, :])
                    for acc, h in ((accA, hA), (accB, hB)):
                        sp, sc = sum_slot(h, sqb)
                        sA = stage_pool.tile([128, 512], FP, tag="sst",
                                             name="sst")
                        nc.vector.tensor_copy(sA[64:65, :], acc[64:65, :])
                        nc.sync.dma_start(sums_t[sp:sp + 1, sc],
                                          sA[64:65, :])
                    # reciprocal for this unit (both heads share a
                    # 64-partition band and column slot)
                    band = 32 * (hA % 4)
                    _, sc = sum_slot(hA, sqb)
                    nc.vector.reciprocal(
                        recip_t[band:band + 64, sc],
                        sums_t[band:band + 64, sc])
                    # normalize y^T for this unit in place
                    for h2, h in ((0, hA), (1, hB)):
                        rows = slice(64 * h2, 64 * h2 + 64)
                        sp, _ = sum_slot(h, sqb)
                        # HW partition_broadcast reads partition 0 of the
                        # tensor regardless of the AP base -> stage the
                        # recip row to partition 0 (cast to bf16) first.
                        rtmp = stage_pool.tile([128, 512], FP, tag="rtmp",
                                               name="rtmp")
                        nc.sync.dma_start(rtmp[0:1, :],
                                          recip_t[sp:sp + 1, sc])
                        rtb = stage_pool.tile([128, 512], BF, tag="rtb",
                                              name="rtb")
                        nc.vector.tensor_copy(rtb[0:1, :], rtmp[0:1, :])
                        rb = stage_pool.tile([128, 512], BF, tag="rb",
                                             name="rb")
                        nc.gpsimd.partition_broadcast(rb[:], rtb[0:1, :])
                        nc.vector.tensor_mul(
                            yT[hp][rows, sq], yT[hp][rows, sq],
                            rb[rows, :])
                    # ship + AllGather this (head-pair, quarter) now
                    nc.sync.dma_start(y_bnc[hp][sqb][:, :], yT[hp][:, sq])
                    nc.gpsimd.collective_compute(
                        "AllGather", mybir.AluOpType.bypass,
                        replica_groups=[[0, 1], [2, 3], [4, 5], [6, 7]],
                        ins=[y_bnc[hp][sqb][:, :]],
                        outs=[y_gath[hp][sqb][:, :]])

            # ---- tail: hp3 row-blocks of the output projection ----
            # tail DMAs trigger from the Scalar queue (idle after the last
            # exp) so the Sync queue stays clear for the final units'
            # normalize -> ship -> AllGather chains
            for q in range(SQB):
                load_ygq(q, (3, 7), eng=nc.scalar)
                for tl in range(4):
                    ti = 4 * q + tl
                    po = mm_psum.tile([128, DQ], FP, tag="mm", name="mm")
                    for j, k2 in enumerate(PB_K2):
                        nc.tensor.matmul(
                            po[:], lhsT=ygq[(k2, q)][:, ts(tl, 128)],
                            rhs=wot3_ref[0][:, k2, :],
                            start=(j == 0), stop=(j == len(PB_K2) - 1))
                    ot = out_stage_ref[0].tile([128, DQ], FP, tag="ot",
                                               name="ot")
                    nc.vector.tensor_add(ot[:], po[:], p1(ti))
                    nc.scalar.dma_start(out_ext[ts(ti, 128), :], ot[:])


def build_program(S=2048):
    nc = bacc.Bacc(
        "TRN2",
        target_bir_lowering=False,
        debug=False,
        enable_asserts=True,
        num_devices=NCORES,
    )
    io = {
        "xt": nc.declare_dram_parameter("xt", [D, S], BF, isOutput=False),
        "wq": nc.declare_dram_parameter("wq", [D, DQ], BF, isOutput=False),
        "bqk": nc.declare_dram_parameter("bqk", [128, 8], FP, isOutput=False),
        "wk": nc.declare_dram_parameter("wk", [D, DQ], BF, isOutput=False),
        "wv": nc.declare_dram_parameter("wv", [D, DQ], BF, isOutput=False),
        "bv": nc.declare_dram_parameter("bv", [DQ], FP, isOutput=False),
        "wo": nc.declare_dram_parameter("wo", [D, DQ], BF, isOutput=False),
        "bo": nc.declare_dram_parameter("bo", [DQ], FP, isOutput=False),
        "out": nc.declare_dram_parameter("out", [S, DQ], FP, isOutput=True),
    }
    io = {k: (v[:] if not isinstance(v, bass.AP) else v) for k, v in io.items()}
    with tile.TileContext(nc) as tc:
        emit_mha(nc, tc, io, S)
    nc.finalize()
    return nc


def shard_inputs(x, Wq, bq, Wk, bk, Wv, bv, Wo, bo):
    """Full inputs -> per-core in_maps. Matmul operands cast to bf16; x is
    transposed on the host (input prep for the d-major device layout)."""
    BFNP = ml_dtypes.bfloat16
    f32 = lambda a: np.ascontiguousarray(np.asarray(a), dtype=np.float32)
    bf = lambda a: np.ascontiguousarray(np.asarray(a, dtype=np.float32)
                                        .astype(BFNP))
    x = np.asarray(x, dtype=np.float32).astype(BFNP)
    xts = [np.ascontiguousarray(x[b].T) for b in range(4)]
    Wq, Wk, Wv, Wo = bf(Wq), bf(Wk), bf(Wv), bf(Wo)
    bq, bk, bv, bo = f32(bq), f32(bk), f32(bv), f32(bo)
    in_maps = []
    for c in range(NCORES):
        b, g = divmod(c, 2)
        sl = slice(g * DQ, (g + 1) * DQ)
        bqk = np.empty((128, 8), np.float32)
        for m in range(4):
            bqk[:, m] = bq[sl][m * 128:(m + 1) * 128]
            bqk[:, 4 + m] = bk[sl][m * 128:(m + 1) * 128]
        in_maps.append({
            "xt": xts[b],
            "wq": np.ascontiguousarray(Wq[:, sl]), "bqk": bqk,
            "wk": np.ascontiguousarray(Wk[:, sl]),
            "wv": np.ascontiguousarray(Wv[:, sl]), "bv": bv[sl].copy(),
            "wo": np.ascontiguousarray(Wo[:, sl]), "bo": bo[sl].copy(),
        })
    return in_maps


_CACHE = {}


def _get_program(S=2048):
    if S not in _CACHE:
        _CACHE[S] = build_program(S)
    return _CACHE[S]


def kernel(x, Wq, bq, Wk, bk, Wv, bv, Wo, bo):
    nc = _get_program(2048)
    in_maps = shard_inputs(x, Wq, bq, Wk, bk, Wv, bv, Wo, bo)
    res = run_bass_kernel_spmd(nc, in_maps, list(range(NCORES))).results
    S = 2048
    out = np.empty((4, S, D), dtype=np.float32)
    for c in range(NCORES):
        b, g = divmod(c, 2)
        out[b, :, g * DQ:(g + 1) * DQ] = res[c]["out"]
    return out


# revision 15
# speedup vs baseline: 1.2035x; 1.1877x over previous
"""Trainium2 Bass kernel: multi-head attention (b=4, s=2048, d_model=1024, h=16).

Sharding over 8 NeuronCores: 2-D (batch x head-half).
  core c -> batch c//2, head group c%2 (8 of 16 heads, qkv dims 512*g..512*g+512).
Per core: QKV column-parallel, per-head attention (scores computed transposed,
softmax sums via a ones-column appended to V in the PV matmul, max-subtraction
skipped -- scores are O(5) so exp is safe), a per-(head-pair, seq-quarter)
AllGather of the normalized per-head outputs, and a column-parallel output
projection computed as three passes of partial sums.

The scalar-engine exp stream (~285us) is the critical path; everything else is
arranged around it:
  - Batched startup DMAs (few triggers) + minimal upfront projection groups
    -> the first exp fires ~12us in.
  - All QKV projection groups are dripped as whole psum-groups at paced drop
    points inside the hp0/hp1 attention units; x/weights are freed at hp2.
  - The output projection is split by y row-blocks so each pass only depends
    on AllGathers that are a full head-pair old (the Tile scheduler models
    collectives pessimistically and defers anything depending on a recent
    one): pass A (hp0+hp1 blocks) drips during hp2's units into a bf16
    partial (reusing the retired qT0/kT0/qT1/kT1 SBUF, bias folded in),
    pass A2 (hp2 blocks) accumulates during hp3's units 0-2, and only
    pass B (hp3 blocks + final add) remains after the last exp.

All matmul operands are bf16 (fp32 PSUM accumulation). The host pre-transposes
x to x^T [D, S] and casts to bf16. The two score matmuls per (head-pair, k)
are row-tiled (tile_position auto-derived from base partitions 0/64) and run
concurrently on the PE.

Host assembly: out[b] = concat(core 2b cols 0:512, core 2b+1 cols 512:1024).

Self-contained: hardcodes all shapes; builds/compiles once per process.
"""

from contextlib import ExitStack

import ml_dtypes
import numpy as np

import concourse.bass as bass
import concourse.mybir as mybir
import concourse.tile as tile
from concourse import bacc
from concourse.bass_utils import run_bass_kernel_spmd

FP = mybir.dt.float32
BF = mybir.dt.bfloat16
AFT = mybir.ActivationFunctionType
ts = bass.ts

NCORES = 8
D = 1024           # d_model
HD = 64            # head dim
HPC = 8            # heads per core
DQ = HPC * HD      # per-core qkv width = 512
SCALE = 1.0 / np.sqrt(HD)


def emit_mha(nc, tc, io, S):
    """Emit the per-core MHA program. io: dict of DRAM APs."""
    NHP = HPC // 2       # head pairs = 4
    KT = S // 128        # sk tiles = 16
    SQB = S // 512       # sq blocks of 512 = 4
    DKT = D // 128       # d_in tiles = 8
    MQ = DQ // 128       # qkv dout tiles = 4
    TT = S // 128        # token tiles = 16

    xt_in, wq_in, bqk_in, wk_in, wv_in, bv_in, wo_in, bo_in, out_ext = (
        io["xt"], io["wq"], io["bqk"], io["wk"], io["wv"], io["bv"],
        io["wo"], io["bo"], io["out"])

    # DRAM sources viewed [part, k-tile, cols]
    xt_r = xt_in.rearrange("(k p) s -> p k s", p=128)
    wq_r = wq_in.rearrange("(k p) c -> p k c", p=128)
    wk_r = wk_in.rearrange("(k p) c -> p k c", p=128)
    wv_r = wv_in.rearrange("(k p) c -> p k c", p=128)
    wo_r = wo_in.rearrange("(k p) c -> p k c", p=128)

    with ExitStack() as ctx:
        const_pool = ctx.enter_context(tc.tile_pool(name="const", bufs=1))
        dram_pool = ctx.enter_context(tc.tile_pool(name="dram", bufs=1, space="DRAM"))
        # PSUM budget: mm 2 + scores 2x2 + accA 1 + accB 1 = 8 banks
        mm_psum = ctx.enter_context(
            tc.tile_pool(name="mmps", bufs=2, space="PSUM"))
        sc_psum = ctx.enter_context(
            tc.tile_pool(name="scps", bufs=2, space="PSUM"))
        ac_psum = ctx.enter_context(
            tc.tile_pool(name="acps", bufs=1, space="PSUM"))

        # biases for q/k, host-packed [128, 2*MQ]: col m = bq tile m, MQ+m = bk
        bias_qk = const_pool.tile([128, 2 * MQ], FP, tag="bqk", name="bqk")
        nc.sync.dma_start(bias_qk[:], bqk_in[:, :])

        # bv / bo broadcast tiles [128, DQ]
        bv_bc = const_pool.tile([128, DQ], FP, tag="bvbc", name="bvbc")
        bo_bc = const_pool.tile([128, DQ], FP, tag="bobc", name="bobc")
        with tc.tile_pool(name="btmpp", bufs=2) as btmp_pool:
            btmp = btmp_pool.tile([128, DQ], FP, tag="btmp", name="btmp")
            nc.sync.dma_start(
                btmp[0:1, :], bv_in[:].rearrange("(one f) -> one f", one=1))
            nc.gpsimd.partition_broadcast(bv_bc[:], btmp[0:1, :])
            btmp2 = btmp_pool.tile([128, DQ], FP, tag="btmp", name="btmp2")
            nc.sync.dma_start(
                btmp2[0:1, :], bo_in[:].rearrange("(one f) -> one f", one=1))
            nc.gpsimd.partition_broadcast(bo_bc[:], btmp2[0:1, :])

        def sum_slot(h, sqb):
            # unit (h, sqb) -> partition 32*(h%4), cols [(h//4)*SQB + sqb]*512
            return 32 * (h % 4), ts((h // 4) * SQB + sqb, 512)

        # DRAM bounce + per-(head-pair, seq-quarter) AllGather in/out (bf16)
        y_bnc = [[dram_pool.tile([128, 512], BF, tag=f"ybounce{hp}_{q}",
                                 name=f"ybounce{hp}_{q}")
                  for q in range(SQB)]
                 for hp in range(NHP)]
        y_gath = [[dram_pool.tile([256, 512], BF, tag=f"ygather{hp}_{q}",
                                  name=f"ygather{hp}_{q}")
                   for q in range(SQB)]
                  for hp in range(NHP)]

        with ExitStack() as phase12:
            qkv_pool = phase12.enter_context(tc.tile_pool(name="qkv", bufs=1))
            yt_pool = phase12.enter_context(tc.tile_pool(name="yt", bufs=1))
            exp_pool = phase12.enter_context(tc.tile_pool(name="exp", bufs=6))
            stage_pool = phase12.enter_context(tc.tile_pool(name="stage", bufs=3))

            # q^T / k^T, d-major: tile hp holds heads 2hp (parts 0-63), 2hp+1
            qT = [qkv_pool.tile([128, S], BF, tag=f"qT{m}", name=f"qT{m}")
                  for m in range(MQ)]
            kT = [qkv_pool.tile([128, S], BF, tag=f"kT{m}", name=f"kT{m}")
                  for m in range(MQ)]
            # v natural [tok, dout] with a ones column per head
            v_ones = [qkv_pool.tile([128, HPC * (HD + 1)], BF, tag=f"v{t}",
                                    name=f"v{t}")
                      for t in range(TT)]
            # y^T (attention out, d-major, normalized in place per unit)
            yT = [yt_pool.tile([128, S], BF, tag=f"yT{m}", name=f"yT{m}")
                  for m in range(MQ)]
            # softmax sums / reciprocals, packed 32-partition-aligned
            sums_t = yt_pool.tile([128, 2 * SQB * 512], FP, tag="sums",
                                  name="sums")
            recip_t = yt_pool.tile([128, 2 * SQB * 512], FP, tag="recip",
                                   name="recip")
            nc.gpsimd.memset(sums_t[:], 1.0)

            phase01 = ExitStack()
            xtw_pool = phase01.enter_context(tc.tile_pool(name="xtw", bufs=1))
            xTall = xtw_pool.tile([128, DKT * S], BF, tag="xTall",
                                  name="xTall")
            xT3 = xTall[:].rearrange("p (d s) -> p d s", s=S)
            # wq/wk as per-head-pair [128, DKT*128] k-major strips
            wqm = [xtw_pool.tile([128, DKT * 128], BF, tag=f"wqm{m}",
                                 name=f"wqm{m}") for m in range(MQ)]
            wkm = [xtw_pool.tile([128, DKT * 128], BF, tag=f"wkm{m}",
                                 name=f"wkm{m}") for m in range(MQ)]
            wq3 = [t[:].rearrange("p (k c) -> p k c", c=128) for t in wqm]
            wk3 = [t[:].rearrange("p (k c) -> p k c", c=128) for t in wkm]
            wvt = xtw_pool.tile([128, DKT * DQ], BF, tag="wvt", name="wvt")
            wv3 = wvt[:].rearrange("p (k c) -> p k c", c=DQ)

            # ---- DMA order = first-consumption order, batched triggers ----
            for kp in range(2):
                nc.sync.dma_start(wk3[0][:, ts(kp, 4), :],
                                  wk_r[:, ts(kp, 4), ts(0, 128)])
            for kp in range(4):
                nc.sync.dma_start(xT3[:, ts(kp, 2), ts(0, 512)],
                                  xt_r[:, ts(kp, 2), ts(0, 512)])
            for kp in range(2):
                nc.sync.dma_start(wq3[0][:, ts(kp, 4), :],
                                  wq_r[:, ts(kp, 4), ts(0, 128)])
            for kp in range(2):
                nc.sync.dma_start(wv3[:, ts(kp, 4), :],
                                  wv_r[:, ts(kp, 4), :])
            for nb in range(1, SQB):
                for kp in range(4):
                    nc.sync.dma_start(xT3[:, ts(kp, 2), ts(nb, 512)],
                                      xt_r[:, ts(kp, 2), ts(nb, 512)])
            for m in range(1, MQ):
                nc.sync.dma_start(wk3[m][:, :, :], wk_r[:, :, ts(m, 128)])
                nc.sync.dma_start(wq3[m][:, :, :], wq_r[:, :, ts(m, 128)])

            def emit_qk_group(hp, which, nb):
                # one q/k projection psum group for head-pair hp; which: 0=q,
                # 1=k; nb = token block. Evac on DVE (keeps the scalar engine
                # free for the exp stream).
                w3, dstT = ((wq3, qT), (wk3, kT))[which]
                ps = mm_psum.tile([128, 512], FP, tag="mm", name="mm")
                for k in range(DKT):
                    nc.tensor.matmul(
                        ps[:], lhsT=w3[hp][:, k, :], rhs=xT3[:, k, ts(nb, 512)],
                        start=(k == 0), stop=(k == DKT - 1))
                col = which * MQ + hp
                nc.vector.tensor_scalar_add(
                    dstT[hp][:, ts(nb, 512)], ps[:],
                    bias_qk[:, col:col + 1])

            def emit_v(ti):
                ps = mm_psum.tile([128, DQ], FP, tag="mm", name="mm")
                for k in range(DKT):
                    nc.tensor.matmul(
                        ps[:], lhsT=xT3[:, k, ts(ti, 128)], rhs=wv3[:, k, :],
                        start=(k == 0), stop=(k == DKT - 1))
                vt3 = v_ones[ti][:].rearrange("p (h u) -> p h u", u=HD + 1)
                nc.vector.tensor_add(
                    vt3[:, :, 0:HD],
                    ps[:].rearrange("p (h u) -> p h u", u=HD),
                    bv_bc[:].rearrange("p (h u) -> p h u", u=HD))
                nc.gpsimd.memset(vt3[:, :, HD:HD + 1], 1.0)

            # ---- upfront: the two groups unit (0,0) needs to start ----
            emit_qk_group(0, 1, 0)   # k proj, head-pair 0, tokens 0:512
            emit_qk_group(0, 0, 0)   # q proj, head-pair 0, tokens 0:512

            # ---- paced drip schedule: {(hp,sqb): {iter: [fn]}} ----
            drip = {}

            def add_drop(hp, sqb, it, fn):
                drip.setdefault((hp, sqb), {}).setdefault(it, []).append(fn)

            def qk(hp, w, nb):
                return lambda: emit_qk_group(hp, w, nb)

            # remaining hp0 groups in unit (0,0); hp1's during hp0's units
            for i, (w, nb) in enumerate(((1, 1), (1, 2), (1, 3), (0, 1))):
                add_drop(0, 0, 3 * i + 1, qk(0, w, nb))
            sched01 = [
                ((0, 1), ((0, 0, 2), (1, 1, 0), (1, 1, 1))),
                ((0, 2), ((0, 0, 3), (1, 1, 2), (1, 1, 3))),
                ((0, 3), ((1, 0, 0), (1, 0, 1), (1, 0, 2), (1, 0, 3))),
                ((1, 0), ((2, 1, 0), (2, 1, 1), (2, 1, 2), (2, 1, 3))),
                ((1, 1), ((2, 0, 0), (2, 0, 1), (2, 0, 2), (2, 0, 3))),
                ((1, 2), ((3, 1, 0), (3, 1, 1), (3, 1, 2), (3, 1, 3))),
                ((1, 3), ((3, 0, 0), (3, 0, 1), (3, 0, 2), (3, 0, 3))),
            ]
            for (hp, sqb), groups in sched01:
                step = 4 if len(groups) == 4 else 5
                for i, (ghp, w, nb) in enumerate(groups):
                    add_drop(hp, sqb, step * i + 1, qk(ghp, w, nb))

            # ---- output projection: three passes of y row-blocks ----
            # k2 in 0..3 -> rank0 rows of gather hp=k2 (y_full rows 128*k2);
            # k2 in 4..7 -> rank1 rows of gather hp=k2-4.
            wot3_ref = [None]
            ygq_pool_ref = [None]
            out_stage_ref = [None]
            p1t = [None] * MQ      # bf16 partials, reuse retired qT/kT space
            ygq = {}               # (k2, q) -> sbuf tile

            def p1(ti):
                return p1t[ti // 4][:, ts(ti % 4, 512)]

            def load_wo():
                p4_pool = phase12.enter_context(
                    tc.tile_pool(name="p4", bufs=1))
                ygq_pool_ref[0] = phase12.enter_context(
                    tc.tile_pool(name="ygq", bufs=4))
                out_stage_ref[0] = phase12.enter_context(
                    tc.tile_pool(name="outp", bufs=3))
                wot = p4_pool.tile([128, 2 * MQ * DQ], BF, tag="wot",
                                   name="wot")
                wot3_ref[0] = wot[:].rearrange("p (k c) -> p k c", c=DQ)
                for kp in range(2):
                    nc.sync.dma_start(wot3_ref[0][:, ts(kp, 4), :],
                                      wo_r[:, ts(kp, 4), :])
                for g in range(MQ):
                    tag = (f"qT{g // 2}", f"kT{g // 2}")[g % 2]
                    p1t[g] = qkv_pool.tile([128, S], BF, tag=tag,
                                           name=f"p1_{g}")

            def load_ygq(q, k2s, eng=None):
                for k2 in k2s:
                    t = ygq_pool_ref[0].tile([128, 512], BF, tag=f"yg{k2}",
                                             name=f"yg{k2}_{q}")
                    src = y_gath[k2 % MQ][q]
                    half = slice(0, 128) if k2 < MQ else slice(128, 256)
                    (eng or nc.sync).dma_start(t[:], src[half, :])
                    ygq[(k2, q)] = t

            def outproj_pass(ti, k2s, first):
                po = mm_psum.tile([128, DQ], FP, tag="mm", name="mm")
                for j, k2 in enumerate(k2s):
                    nc.tensor.matmul(
                        po[:], lhsT=ygq[(k2, ti // 4)][:, ts(ti % 4, 128)],
                        rhs=wot3_ref[0][:, k2, :],
                        start=(j == 0), stop=(j == len(k2s) - 1))
                if first:
                    nc.vector.tensor_add(p1(ti), po[:], bo_bc[:])
                else:
                    nc.vector.tensor_add(p1(ti), p1(ti), po[:])

            PA_K2, PB_K2 = (0, 1, 4, 5), (2, 3, 6, 7)
            for q in range(SQB):
                add_drop(2, q, 0, lambda q=q: load_ygq(q, PA_K2))
                for tl in range(4):
                    add_drop(2, q, 4 * tl + 1,
                             lambda ti=4 * q + tl:
                                 outproj_pass(ti, PA_K2, True))
            # prefetch hp2's gathered-y quarters during hp3 (DMA only; the
            # matmuls run in the tail pass to keep hp3's DVE queue clean)
            for q in range(SQB):
                add_drop(3, min(q, 2), 4 * q + 1,
                         lambda q=q: load_ygq(q, (2, 6)))

            # ---- attention units ----
            for hp in range(NHP):
                if hp == 2:
                    # all projection drips done -> free x/weights, bring in Wo
                    phase01.close()
                    load_wo()

                hA, hB = 2 * hp, 2 * hp + 1
                for sqb in range(SQB):
                    sq = ts(sqb, 512)
                    drops = drip.get((hp, sqb), {})
                    for fn in drops.get(-1, ()):
                        fn()
                    accA = ac_psum.tile([HD + 1, 512], FP, tag="accA",
                                        name="accA")
                    accB = ac_psum.tile([HD + 1, 512], FP, tag="accB",
                                        name="accB")

                    def emit_scores(k):
                        sk = ts(k, 128)
                        ps = sc_psum.tile([128, 1024], FP, tag="sc",
                                          name="sc")
                        # scores^T [sk, sq] for both heads; base partitions
                        # 0/64 -> row-tiled, the matmuls run concurrently
                        nc.tensor.matmul(
                            ps[:, 0:512], lhsT=kT[hp][0:64, sk],
                            rhs=qT[hp][0:64, sq], start=True, stop=True)
                        nc.tensor.matmul(
                            ps[:, 512:1024], lhsT=kT[hp][64:128, sk],
                            rhs=qT[hp][64:128, sq], start=True, stop=True)
                        et = exp_pool.tile([128, 1024], BF, tag="exp",
                                           name="exp")
                        nc.scalar.activation(et[:], ps[:], AFT.Exp,
                                             scale=SCALE)
                        if hp == 0 and sqb == 0:
                            # produce v[k] just in time for its attnv
                            emit_v(k)
                        return et

                    def emit_av(k, et):
                        # y^T accumulation: lhsT = [v_h | 1]
                        nc.tensor.matmul(
                            accA[:], lhsT=v_ones[k][:, hA * 65:hA * 65 + 65],
                            rhs=et[:, 0:512],
                            start=(k == 0), stop=(k == KT - 1),
                            skip_group_check=True)
                        nc.tensor.matmul(
                            accB[:], lhsT=v_ones[k][:, hB * 65:hB * 65 + 65],
                            rhs=et[:, 512:1024],
                            start=(k == 0), stop=(k == KT - 1),
                            skip_group_check=True)
                        for fn in drops.get(k, ()):
                            fn()

                    # software-pipeline: scores run one iteration ahead of
                    # the AV accumulation, so an AV stall (e.g. the previous
                    # unit's accumulator extraction) never blocks the next
                    # score pair in the in-order PE queue -> the exp stream
                    # keeps flowing across unit boundaries.
                    prev_et = None
                    for k in range(KT):
                        et = emit_scores(k)
                        if prev_et is not None:
                            emit_av(k - 1, prev_et)
                        prev_et = et
                    emit_av(KT - 1, prev_et)
                    # extract y (rows 0-63) and sums (row 64)
                    nc.vector.tensor_copy(yT[hp][0:64, sq], accA[0:64, :])
                    st = stage_pool.tile([128, 512], BF, tag="bst", name="bst")
                    nc.vector.tensor_copy(st[0:64, :], accB[0:64, :])
                    nc.sync.dma_start(yT[hp][64:128, sq], st[0:64, :])
                    for acc, h in ((accA, hA), (accB, hB)):
                        sp, sc = sum_slot(h, sqb)
                        sA = stage_pool.tile([128, 512], FP, tag="sst",
                                             name="sst")
                        nc.vector.tensor_copy(sA[64:65, :], acc[64:65, :])
                        nc.sync.dma_start(sums_t[sp:sp + 1, sc],
                                          sA[64:65, :])
                    # reciprocal for this unit (both heads share a
                    # 64-partition band and column slot)
                    band = 32 * (hA % 4)
                    _, sc = sum_slot(hA, sqb)
                    nc.vector.reciprocal(
                        recip_t[band:band + 64, sc],
                        sums_t[band:band + 64, sc])
                    # normalize y^T for this unit in place
                    for h2, h in ((0, hA), (1, hB)):
                        rows = slice(64 * h2, 64 * h2 + 64)
                        sp, _ = sum_slot(h, sqb)
                        # HW partition_broadcast reads partition 0 of the
                        # tensor regardless of the AP base -> stage the
                        # recip row to partition 0 (cast to bf16) first.
                        rtmp = stage_pool.tile([128, 512], FP, tag="rtmp",
                                               name="rtmp")
                        nc.sync.dma_start(rtmp[0:1, :],
                                          recip_t[sp:sp + 1, sc])
                        rtb = stage_pool.tile([128, 512], BF, tag="rtb",
                                              name="rtb")
                        nc.vector.tensor_copy(rtb[0:1, :], rtmp[0:1, :])
                        rb = stage_pool.tile([128, 512], BF, tag="rb",
                                             name="rb")
                        nc.gpsimd.partition_broadcast(rb[:], rtb[0:1, :])
                        nc.vector.tensor_mul(
                            yT[hp][rows, sq], yT[hp][rows, sq],
                            rb[rows, :])
                    # ship + AllGather this (head-pair, quarter) now
                    nc.sync.dma_start(y_bnc[hp][sqb][:, :], yT[hp][:, sq])
                    nc.gpsimd.collective_compute(
                        "AllGather", mybir.AluOpType.bypass,
                        replica_groups=[[0, 1], [2, 3], [4, 5], [6, 7]],
                        ins=[y_bnc[hp][sqb][:, :]],
                        outs=[y_gath[hp][sqb][:, :]])

            # ---- tail: hp3 row-blocks of the output projection ----
            # tail DMAs trigger from the Scalar queue (idle after the last
            # exp) so the Sync queue stays clear for the final units'
            # normalize -> ship -> AllGather chains
            for q in range(SQB):
                load_ygq(q, (3, 7), eng=nc.scalar)
                for tl in range(4):
                    ti = 4 * q + tl
                    po = mm_psum.tile([128, DQ], FP, tag="mm", name="mm")
                    for j, k2 in enumerate(PB_K2):
                        nc.tensor.matmul(
                            po[:], lhsT=ygq[(k2, q)][:, ts(tl, 128)],
                            rhs=wot3_ref[0][:, k2, :],
                            start=(j == 0), stop=(j == len(PB_K2) - 1))
                    ot = out_stage_ref[0].tile([128, DQ], FP, tag="ot",
                                               name="ot")
                    nc.vector.tensor_add(ot[:], po[:], p1(ti))
                    nc.scalar.dma_start(out_ext[ts(ti, 128), :], ot[:])


def build_program(S=2048):
    nc = bacc.Bacc(
        "TRN2",
        target_bir_lowering=False,
        debug=False,
        enable_asserts=False,
        num_devices=NCORES,
    )
    io = {
        "xt": nc.declare_dram_parameter("xt", [D, S], BF, isOutput=False),
        "wq": nc.declare_dram_parameter("wq", [D, DQ], BF, isOutput=False),
        "bqk": nc.declare_dram_parameter("bqk", [128, 8], FP, isOutput=False),
        "wk": nc.declare_dram_parameter("wk", [D, DQ], BF, isOutput=False),
        "wv": nc.declare_dram_parameter("wv", [D, DQ], BF, isOutput=False),
        "bv": nc.declare_dram_parameter("bv", [DQ], FP, isOutput=False),
        "wo": nc.declare_dram_parameter("wo", [D, DQ], BF, isOutput=False),
        "bo": nc.declare_dram_parameter("bo", [DQ], FP, isOutput=False),
        "out": nc.declare_dram_parameter("out", [S, DQ], FP, isOutput=True),
    }
    io = {k: (v[:] if not isinstance(v, bass.AP) else v) for k, v in io.items()}
    with tile.TileContext(nc) as tc:
        emit_mha(nc, tc, io, S)
    nc.finalize()
    return nc


def shard_inputs(x, Wq, bq, Wk, bk, Wv, bv, Wo, bo):
    """Full inputs -> per-core in_maps. Matmul operands cast to bf16; x is
    transposed on the host (input prep for the d-major device layout)."""
    BFNP = ml_dtypes.bfloat16
    f32 = lambda a: np.ascontiguousarray(np.asarray(a), dtype=np.float32)
    bf = lambda a: np.ascontiguousarray(np.asarray(a, dtype=np.float32)
                                        .astype(BFNP))
    x = np.asarray(x, dtype=np.float32).astype(BFNP)
    xts = [np.ascontiguousarray(x[b].T) for b in range(4)]
    Wq, Wk, Wv, Wo = bf(Wq), bf(Wk), bf(Wv), bf(Wo)
    bq, bk, bv, bo = f32(bq), f32(bk), f32(bv), f32(bo)
    in_maps = []
    for c in range(NCORES):
        b, g = divmod(c, 2)
        sl = slice(g * DQ, (g + 1) * DQ)
        bqk = np.empty((128, 8), np.float32)
        for m in range(4):
            bqk[:, m] = bq[sl][m * 128:(m + 1) * 128]
            bqk[:, 4 + m] = bk[sl][m * 128:(m + 1) * 128]
        in_maps.append({
            "xt": xts[b],
            "wq": np.ascontiguousarray(Wq[:, sl]), "bqk": bqk,
            "wk": np.ascontiguousarray(Wk[:, sl]),
            "wv": np.ascontiguousarray(Wv[:, sl]), "bv": bv[sl].copy(),
            "wo": np.ascontiguousarray(Wo[:, sl]), "bo": bo[sl].copy(),
        })
    return in_maps


_CACHE = {}


def _get_program(S=2048):
    if S not in _CACHE:
        _CACHE[S] = build_program(S)
    return _CACHE[S]


def kernel(x, Wq, bq, Wk, bk, Wv, bv, Wo, bo):
    nc = _get_program(2048)
    in_maps = shard_inputs(x, Wq, bq, Wk, bk, Wv, bv, Wo, bo)
    res = run_bass_kernel_spmd(nc, in_maps, list(range(NCORES))).results
    S = 2048
    out = np.empty((4, S, D), dtype=np.float32)
    for c in range(NCORES):
        b, g = divmod(c, 2)
        out[b, :, g * DQ:(g + 1) * DQ] = res[c]["out"]
    return out
